# revision 7
# baseline (speedup 1.0000x reference)
"""CliffordBatchNormMV Trainium2 kernel.

Math (per grade g, block nb, batch token b):
  sumsq[b,nb,g] = sum_{c in grade g} x[b,nb,c]^2
  n = sqrt(sumsq + EPS)                       # grade norm
  mean/var over b (biased)                    # batch stats per (g, nb)
  inv = 1/sqrt(var + EPS)
  out[c] = x[c] * (A[g,nb] + C[g,nb] / n),  A = gs*gamma*inv,
                                            C = gs*(beta - gamma*inv*mean)

Distribution: shard the 64 nb-blocks across 8 cores (8 each) -> batch stats
are fully core-local, no collectives.

Per-core layout: host pre-transposes the shard to [8, 4096, 256] (nb-major).
Each nb-group (4096 tokens) streams through in 512-token chunks (4 tiles of
[128 tok, 256 mv]; token t of group = p*32 + q for partition p, slot q;
2 MiB DMAs cover 4 chunks):
  pass 1 per chunk: PE-transpose x -> PSUM [mv, tok], ACT Square -> x2T
    (f32r), f32r matmul against the 0/1 grade-membership matrix G ->
    sumsq [9, 512] PSUM (and a second accumulating matmul pair into a
    persistent stats bank = sum over the batch of sumsq), ACT Sqrt
    (+accum_out = sum of norms) -> gnorm, DVE reciprocal_approx_fast ->
    rgnorm (kept for pass 2).
  stats per group: E[n] from the sqrt accums, E[n^2] from the stats bank;
    var = E[n^2] - mean^2 (+EPS folded into the Sqrt bias); inv via
    Sqrt + reciprocal_approx_fast; A = gg*inv, C = gb - A*mean.
  pass 2 per chunk: s = C*rgnorm + A (DVE tensor_scalar, f32r), f32r
    matmul with G9 [9, 256] expands s to [128 tok, 256 mv] PSUM, DVE
    multiplies in-place into the resident x tile, DMA out.

All engines stay below the ~187 us/core DMA floor (64 MiB I/O at
~360 GB/s); cost-model timeline estimate ~215 us/core.
"""

import os
import numpy as np

MV = 256
NG = 9
EPS = 1e-5
B = 4096
NB = 64
N_CORES = 8
NB_PER_CORE = NB // N_CORES      # 8 nb-groups per core
QS = 32                          # token slots per partition per group
NCHUNK = 8                       # chunks per group
TPC = 4                          # token-tiles per chunk (512 tokens)

_GRADES = np.array([bin(i).count("1") for i in range(MV)])

LAST_RESULTS = None
_CACHE = {}


def _build_program():
    import concourse.bacc as bacc
    import concourse.tile as tile
    from concourse import mybir

    f32 = mybir.dt.float32
    f32r = mybir.dt.float32r
    AF = mybir.ActivationFunctionType
    Alu = mybir.AluOpType

    G_full = np.zeros((MV, NG), dtype=np.float32)
    G_full[np.arange(MV), _GRADES] = 1.0
    ident_np = np.eye(128, dtype=np.float32)

    nc = bacc.Bacc()
    x_in = nc.dram_tensor("x", [NB_PER_CORE, B, MV], f32, kind="ExternalInput")
    gg_in = nc.dram_tensor("gg", [NG, NB_PER_CORE], f32, kind="ExternalInput")
    gb_in = nc.dram_tensor("gb", [NG, NB_PER_CORE], f32, kind="ExternalInput")
    out_d = nc.dram_tensor("out", [NB_PER_CORE, B, MV], f32, kind="ExternalOutput")

    G_lo_c = nc.inline_tensor(G_full[:128], name="Glo")
    G_hi_c = nc.inline_tensor(G_full[128:], name="Ghi")
    G9_c = nc.inline_tensor(np.ascontiguousarray(G_full.T), name="G9")
    I_c = nc.inline_tensor(ident_np, name="Ident")

    inv_B = 1.0 / B

    with tile.TileContext(nc) as tc:
        with (
            tc.tile_pool(name="const", bufs=1) as const,
            tc.tile_pool(name="xc", bufs=int(os.environ.get("K_XC", "7"))) as xcp,
            tc.tile_pool(name="work", bufs=int(os.environ.get("K_WORK", "3"))) as work,
            tc.tile_pool(name="grp", bufs=int(os.environ.get("K_GRP", "2"))) as grp,
            tc.tile_pool(name="statp", bufs=2) as statp,
            tc.tile_pool(name="ps_xt", bufs=int(os.environ.get("K_PSXT", "2")), space="PSUM") as ps_xt,
            tc.tile_pool(name="ps_s", bufs=int(os.environ.get("K_PSS", "1")), space="PSUM") as ps_s,
            tc.tile_pool(name="ps_st", bufs=int(os.environ.get("K_PSST", "1")), space="PSUM") as ps_st,
            tc.tile_pool(name="ps_a", bufs=int(os.environ.get("K_PSA", "2")), space="PSUM") as ps_a,
        ):
            Glo = const.tile([128, NG], f32r)
            nc.gpsimd.dma_start(out=Glo, in_=G_lo_c[:, :])
            Ghi = const.tile([128, NG], f32r)
            nc.gpsimd.dma_start(out=Ghi, in_=G_hi_c[:, :])
            G9 = const.tile([NG, MV], f32r)
            nc.gpsimd.dma_start(out=G9, in_=G9_c[:, :])
            ident = const.tile([128, 128], f32)
            nc.sync.dma_start(out=ident, in_=I_c[:, :])
            gg = const.tile([NG, NB_PER_CORE], f32)
            nc.sync.dma_start(out=gg, in_=gg_in[:, :])
            gb = const.tile([NG, NB_PER_CORE], f32)
            nc.sync.dma_start(out=gb, in_=gb_in[:, :])
            eps9 = const.tile([NG, 1], f32)
            nc.vector.memset(eps9, EPS)
            eps2_9 = const.tile([NG, 1], f32)
            nc.vector.memset(eps2_9, 2.0 * EPS)

            for g in range(int(os.environ.get("K_NGROUPS", str(NB_PER_CORE)))):
                xv = x_in[g].rearrange("(p q) c -> p q c", q=QS)
                ov = out_d[g].rearrange("(p q) c -> p q c", q=QS)

                rg = grp.tile([NG, NCHUNK, 512], f32, tag="rg")
                gsum = grp.tile([NG, NCHUNK], f32, tag="gsum")
                st_ps = ps_st.tile([NG, 512], f32, tag="stps")

                xps = []
                CPT = int(os.environ.get("K_CPT", "4"))
                # ---- pass 1: norms + stat accumulators ----
                for ch in range(NCHUNK):
                    if ch % CPT == 0:
                        xp = xcp.tile([128, CPT * TPC, MV], f32, tag="xc")
                        xps.append(xp)
                        nc.sync.dma_start(
                            out=xp, in_=xv[:, ch * TPC:(ch + CPT) * TPC, :]
                        )
                    xc = xps[ch // CPT][:, (ch % CPT) * TPC:(ch % CPT + 1) * TPC, :]
                    xt_lo = ps_xt.tile([128, 512], f32, tag="xtlo")
                    xt_hi = ps_xt.tile([128, 512], f32, tag="xthi")
                    for k in range(TPC):
                        nc.tensor.transpose(
                            xt_lo[:, k * 128:(k + 1) * 128], xc[:, k, 0:128], ident
                        )
                        nc.tensor.transpose(
                            xt_hi[:, k * 128:(k + 1) * 128], xc[:, k, 128:256], ident
                        )
                    x2lo = work.tile([128, 512], f32r, tag="x2lo")
                    nc.scalar.activation(out=x2lo, in_=xt_lo, func=AF.Square)
                    x2hi = work.tile([128, 512], f32r, tag="x2hi")
                    nc.scalar.activation(out=x2hi, in_=xt_hi, func=AF.Square)

                    ps = ps_s.tile([NG, 512], f32, tag="ps")
                    nc.tensor.matmul(ps, Glo, x2lo, start=True, stop=False)
                    nc.tensor.matmul(ps, Ghi, x2hi, start=False, stop=True)
                    # accumulate sum over all tokens of sumsq into stats bank
                    nc.tensor.matmul(st_ps, Glo, x2lo,
                                     start=(ch == 0), stop=False)
                    nc.tensor.matmul(st_ps, Ghi, x2hi,
                                     start=False, stop=(ch == NCHUNK - 1))

                    if ch % 2 == 0:
                        gn2 = work.tile([NG, 2, 512], f32, tag="gn2")
                    nc.scalar.activation(
                        out=gn2[:, ch % 2, :], in_=ps, func=AF.Sqrt,
                        bias=eps9[:, 0:1], accum_out=gsum[:, ch:ch + 1],
                    )
                    if ch % 2 == 1:
                        nc.vector.reciprocal_approx_fast(
                            out=rg[:, ch - 1:ch + 1, :].rearrange("p a b -> p (a b)"),
                            in_=gn2.rearrange("p a b -> p (a b)"),
                        )

                # ---- batch stats -> A, C ----
                en2 = statp.tile([NG, 1], f32, tag="en2")
                nc.vector.tensor_reduce(
                    out=en2, in_=st_ps, axis=mybir.AxisListType.X, op=Alu.add
                )
                mn = statp.tile([NG, 1], f32, tag="mn")   # -mean
                nc.vector.tensor_reduce(
                    out=mn, in_=gsum, axis=mybir.AxisListType.X, op=Alu.add
                )
                nc.vector.tensor_scalar(
                    out=mn, in0=mn, scalar1=-inv_B, scalar2=None, op0=Alu.mult
                )
                m2 = statp.tile([NG, 1], f32, tag="m2")
                nc.vector.tensor_mul(m2, mn, mn)
                # var + EPS = en2/B + 2*EPS - mean^2   (en2 lacks the +EPS)
                var = statp.tile([NG, 1], f32, tag="var")
                nc.vector.scalar_tensor_tensor(
                    out=var, in0=en2, scalar=inv_B, in1=m2,
                    op0=Alu.mult, op1=Alu.subtract,
                )
                sd = statp.tile([NG, 1], f32, tag="sd")
                nc.scalar.activation(
                    out=sd, in_=var, func=AF.Sqrt, bias=eps2_9[:, 0:1]
                )
                inv = statp.tile([NG, 1], f32, tag="inv")
                nc.vector.reciprocal_approx_fast(out=inv, in_=sd)
                A = statp.tile([NG, 1], f32, tag="A")
                nc.vector.tensor_mul(A, gg[:, g:g + 1], inv)
                C = statp.tile([NG, 1], f32, tag="C")
                nc.vector.scalar_tensor_tensor(
                    out=C, in0=A, scalar=mn, in1=gb[:, g:g + 1],
                    op0=Alu.mult, op1=Alu.add,
                )

                # ---- pass 2: apply ----
                for ch in range(NCHUNK):
                    xc = xps[ch // CPT][:, (ch % CPT) * TPC:(ch % CPT + 1) * TPC, :]
                    if ch % 2 == 0:
                        s2 = work.tile([NG, 2, 512], f32r, tag="s2")
                        s_eng = nc.gpsimd if os.environ.get("K_SPOOL", "0") == "1" else nc.vector
                        s_eng.tensor_scalar(
                            out=s2.rearrange("p a b -> p (a b)"),
                            in0=rg[:, ch:ch + 2, :].rearrange("p a b -> p (a b)"),
                            scalar1=C, scalar2=A,
                            op0=Alu.mult, op1=Alu.add,
                        )
                    s = s2[:, ch % 2, :]
                    for half in range(2):
                        pa = ps_a.tile([128, 2, MV], f32, tag="pa")
                        for j in range(2):
                            k = half * 2 + j
                            nc.tensor.matmul(
                                pa[:, j, :], s[:, k * 128:(k + 1) * 128], G9,
                                start=True, stop=True,
                            )
                        q0 = half * 2
                        nc.vector.tensor_mul(
                            xc[:, q0:q0 + 2, :], xc[:, q0:q0 + 2, :], pa
                        )
                    if g == NB_PER_CORE - 1:
                        nc.sync.dma_start(
                            out=ov[:, ch * TPC:(ch + 1) * TPC, :], in_=xc
                        )
                    elif ch % CPT == CPT - 1:
                        nc.sync.dma_start(
                            out=ov[:, (ch - CPT + 1) * TPC:(ch + 1) * TPC, :],
                            in_=xps[ch // CPT],
                        )

    nc.compile()
    return nc


def kernel(x, gamma, beta, grade_scale):
    global LAST_RESULTS
    from concourse.bass_utils import run_bass_kernel_spmd

    if "nc" not in _CACHE:
        _CACHE["nc"] = _build_program()
    nc = _CACHE["nc"]

    x = np.asarray(x)
    assert x.shape == (B, NB, MV) and x.dtype == np.float32, (x.shape, x.dtype)
    gamma = np.asarray(gamma, dtype=np.float32)
    beta = np.asarray(beta, dtype=np.float32)
    grade_scale = np.asarray(grade_scale, dtype=np.float32)

    gg = grade_scale[:, None] * gamma          # [9, 64]
    gb = grade_scale[:, None] * beta           # [9, 64]

    x_t = x.transpose(1, 0, 2)                 # [64, 4096, 256] (view)
    in_maps = []
    for i in range(N_CORES):
        sl = slice(i * NB_PER_CORE, (i + 1) * NB_PER_CORE)
        in_maps.append({
            "x": np.ascontiguousarray(x_t[sl]),
            "gg": np.ascontiguousarray(gg[:, sl]),
            "gb": np.ascontiguousarray(gb[:, sl]),
        })

    want_trace = bool(int(os.environ.get("KERNEL_TRACE", "0") or "0"))
    if want_trace:
        # tracing under axon needs the NTFF hook; fall back cleanly if absent
        try:
            from antenv.axon_hooks import get_axon_ntff_profile_hook
            want_trace = get_axon_ntff_profile_hook() is not None
        except Exception:
            want_trace = False
    res = run_bass_kernel_spmd(
        nc, in_maps, core_ids=list(range(N_CORES)), trace=want_trace,
    )
    LAST_RESULTS = res

    out_t = np.concatenate([res.results[i]["out"] for i in range(N_CORES)], axis=0)
    out = np.ascontiguousarray(out_t.transpose(1, 0, 2)).astype(np.float32, copy=False)
    return out

